# revision 20
# baseline (speedup 1.0000x reference)
"""Sparse (graph-edge) multi-head attention block on 8 TRN2 NeuronCores.

Problem: nn_MultiHeadAttention_6966436954266
  B=2, N=20000, D=256, H=8, dh=32, E=160000 (8 out-edges per node, sorted by src)

  xn  = LN1(x); q,k,v = xn @ w{q,k,v}; per-edge w = exp(q_src.k_dst/sqrt(dh))
  attn = segment_sum(w*v_dst)/segment_sum(w); concat = xn + attn
  out = relu(LN2(concat) @ wo + bo) + concat

Sharding: nodes partitioned contiguously across 8 cores (2500 each). Each core
LN+projects its shard into an fp8 K/V row table (row = [K0 V0 K1 V1], 1KB),
AllGathered into one Shared 20MB table; each core then runs the edge stage for
its own nodes' edges (grouped by src, 8 slots/node).

Edge-stage per 128-node block (8 tiles of 16 nodes x 8 slots):
  - ONE dma_gather per 512 edge slots fetches the full 1KB row (K and V for
    both batches) -> [128, 4, 1024] fp8. SWDGE descriptor emission (~8.6ns per
    descriptor, measured) is the gather bottleneck, so descriptor count is
    minimized by fetching everything an edge needs in one descriptor.
  - q rows are broadcast node->edge-slots with a constant [16,128] selector
    matmul on PE (out = M16^T @ q_tile), PSUM f32.
  - qk product on DVE (fp8 K upconverts in-op), per-head logits via strided
    tensor_reduce, exp on ACT (wq/bq are pre-scaled by 1/sqrt(dh) on host).
  - w*v on DVE with free-dim broadcast; segment sum via PE (lhsT = constant
    0/1 slot-selector), accumulating [128 nodes, 256+8] in PSUM per block.
  - Block epilogue (attn=num/den, concat=xn+attn, LN2 + wo matmul + relu +
    residual + store) runs fused, overlapping later blocks' gathers.
"""

import math
from dataclasses import dataclass

import numpy as np

import concourse.bass as bass
import concourse.bacc as bacc
import concourse.mybir as mybir
import concourse.tile as tile
from concourse.masks import make_identity

B, N, D, H, DH = 2, 20000, 256, 8, 32
NCORES = 8
EPS = 1e-3
P = 128
F32 = mybir.dt.float32
BF16 = mybir.dt.bfloat16
FP8 = mybir.dt.float8e4
I16 = mybir.dt.int16
AF = mybir.ActivationFunctionType
ALU = mybir.AluOpType
SUBT = 4  # tiles per dma_gather (512-index SWDGE ring limit)


@dataclass(frozen=True)
class Cfg:
    n: int = N
    nloc: int = N // NCORES
    ncores: int = NCORES
    b: int = B
    r: int = 8          # edge slots per node (pow2, divides 128)
    mask_all: bool = False
    apply_gb1: bool = False
    apply_gb2: bool = False
    apply_bqkv: bool = False
    apply_bo: bool = False
    swdge_queues: int = 4

    @property
    def npt(self):  # nodes per 128-edge tile
        return P // self.r

    @property
    def nt(self):  # real edge tiles per batch
        return math.ceil(self.nloc / self.npt)

    @property
    def tpb(self):  # tiles per 128-node block
        return P // self.npt

    @property
    def nblk(self):  # 128-node blocks (gather/segment granularity)
        return math.ceil(self.nt / self.tpb)

    @property
    def nt_pad(self):  # idx tiles padded to whole blocks
        return self.nblk * self.tpb

    @property
    def rt(self):  # 128-row tiles per batch (dense stages)
        return math.ceil(self.nloc / P)


def _ceil_div(a, b):
    return (a + b - 1) // b


# ------------------------------------------------------------------- program
def build_program(cfg: Cfg) -> bass.Bass:
    nc = bacc.Bacc("TRN2", num_devices=cfg.ncores,
                   num_swdge_queues=cfg.swdge_queues)
    d = D
    rt = cfg.rt
    nloc, b_ = cfg.nloc, cfg.b

    x_in = nc.dram_tensor("x", [b_ * nloc, d], F32, kind="ExternalInput")
    wqkv_in = nc.dram_tensor("wqkv", [d, 3 * d], BF16, kind="ExternalInput")
    wo_in = nc.dram_tensor("wo", [d, d], BF16, kind="ExternalInput")
    idx_in = nc.dram_tensor("idx", [P, cfg.nt_pad * (P // 16)], I16,
                            kind="ExternalInput")
    mask_in = nc.dram_tensor("mask", [P, cfg.nt_pad], F32, kind="ExternalInput")
    sseg_in = nc.dram_tensor("sseg", [P, cfg.tpb * P], BF16,
                             kind="ExternalInput")
    m8_in = nc.dram_tensor("m8", [P, 8 * P], BF16, kind="ExternalInput")
    gb_in = nc.dram_tensor("gb", [4, d], F32, kind="ExternalInput")
    brow_in = nc.dram_tensor("brow", [1, 4 * d], F32, kind="ExternalInput")
    out_ext = nc.dram_tensor("out", [b_ * nloc, d], F32, kind="ExternalOutput")

    kv_loc = [nc.dram_tensor(f"kv_loc{b}", [nloc, 2 * d], FP8)
              for b in range(b_)]
    kv_full = [nc.dram_tensor(
        f"kv_full{b}", [cfg.n, 2 * d], FP8,
        addr_space="Shared" if cfg.ncores > 4 else "Local")
        for b in range(b_)]

    with tile.TileContext(nc) as tc:
        with tc.tile_pool(name="const", bufs=1) as cpool:
            ident_f = cpool.tile([P, P], F32)
            make_identity(nc, ident_f[:])
            ident_b = cpool.tile([P, P], BF16)
            make_identity(nc, ident_b[:])
            wqkv_sb = cpool.tile([P, 2, 3 * d], BF16)
            nc.sync.dma_start(
                out=wqkv_sb[:],
                in_=wqkv_in[:].rearrange("(ci p) q -> p ci q", p=P))
            wo_sb = cpool.tile([P, 2, d], BF16)
            nc.sync.dma_start(
                out=wo_sb[:], in_=wo_in[:].rearrange("(ci p) q -> p ci q", p=P))
            sseg_sb = cpool.tile([P, cfg.tpb, P], BF16)
            nc.sync.dma_start(
                out=sseg_sb[:],
                in_=sseg_in[:].rearrange("p (k m) -> p k m", k=cfg.tpb))
            m8_sb = cpool.tile([P, 8, P], BF16)
            nc.sync.dma_start(
                out=m8_sb[:], in_=m8_in[:].rearrange("p (k m) -> p k m", k=8))
            idx_sb = cpool.tile([P, cfg.nt_pad * (P // 16)], I16)
            nc.sync.dma_start(out=idx_sb[:], in_=idx_in[:])
            mask_sb = cpool.tile([P, cfg.nt_pad], F32)
            if cfg.mask_all or cfg.nloc % cfg.npt:
                nc.sync.dma_start(out=mask_sb[:], in_=mask_in[:])
            eps_sb = cpool.tile([P, 1], F32)
            nc.vector.memset(eps_sb[:], EPS)
            gb_sb = cpool.tile([1, 4, d], F32)
            if cfg.apply_gb1 or cfg.apply_gb2:
                nc.sync.dma_start(out=gb_sb[:],
                                  in_=gb_in[:].rearrange("g d -> 1 g d"))
            brow_sb = cpool.tile([1, 4 * d], F32)
            if cfg.apply_bqkv or cfg.apply_bo:
                nc.sync.dma_start(out=brow_sb[:], in_=brow_in[:])

            with tc.tile_pool(name="resident", bufs=1) as rpool:
                xn_sb = rpool.tile([P, b_ * rt, d], BF16)
                xnt_sb = rpool.tile([P, 2, b_ * rt, P], BF16)
                q_sb = rpool.tile([P, b_, rt, d], BF16)
                for b in range(b_):
                    _stage1(nc, tc, cfg, x_in, xn_sb, xnt_sb, ident_b, eps_sb,
                            gb_sb, b)
                    _stage2_kv(nc, tc, cfg, xnt_sb, wqkv_sb, brow_sb,
                               kv_loc[b], b)
                    nc.gpsimd.collective_compute(
                        "AllGather",
                        ALU.bypass,
                        replica_groups=[list(range(cfg.ncores))],
                        ins=[kv_loc[b][:]],
                        outs=[kv_full[b][:]],
                    )
                _stage2_q(nc, tc, cfg, xnt_sb, wqkv_sb, brow_sb, q_sb)
                for b in range(b_):
                    _stage4(nc, tc, cfg, idx_sb, kv_full[b], q_sb, xn_sb,
                            sseg_sb, m8_sb, mask_sb, wo_sb, ident_b, eps_sb,
                            gb_sb, brow_sb, out_ext, b)
                    _stage5(nc, tc, cfg, xn_sb, wo_sb, ident_b, eps_sb, gb_sb,
                            brow_sb, out_ext, b)
    nc.finalize()
    return nc


def _layer_norm_rs(nc, pool, src_ap, rows, eps_sb):
    """bn_stats -> mv [P,2] f32 with [:,0]=mean, [:,1]=1/sqrt(var+eps)."""
    stats = pool.tile([P, 6], F32, tag="ln_stats")
    nc.vector.bn_stats(out=stats[:rows], in_=src_ap)
    mv = pool.tile([P, 2], F32, tag="ln_mv")
    nc.vector.bn_aggr(out=mv[:rows], in_=stats[:rows])
    nc.scalar.activation(out=mv[:rows, 1:2], in_=mv[:rows, 1:2], func=AF.Sqrt,
                         bias=eps_sb[:rows], scale=1.0)
    nc.vector.reciprocal(out=mv[:rows, 1:2], in_=mv[:rows, 1:2])
    return mv


def _stage1(nc, tc, cfg, x_in, xn_sb, xnt_sb, ident_b, eps_sb, gb_sb, b):
    d, rt, nloc = D, cfg.rt, cfg.nloc
    with tc.tile_pool(name=f"s1_{b}", bufs=6) as pool, \
         tc.tile_pool(name=f"s1p_{b}", bufs=6, space="PSUM") as ppool:
        if True:
            for irt in range(rt):
                bt = b * rt + irt
                rows = min(P, nloc - irt * P)
                xt = pool.tile([P, d], F32, tag="xt")
                nc.sync.dma_start(
                    out=xt[:rows],
                    in_=x_in[b * nloc + irt * P: b * nloc + irt * P + rows, :])
                if rows < P:
                    nc.vector.memset(xn_sb[:, bt, :], 0.0)
                mv = _layer_norm_rs(nc, pool, xt[:rows], rows, eps_sb)
                nc.vector.tensor_scalar(
                    out=xn_sb[:rows, bt, :], in0=xt[:rows],
                    scalar1=mv[:rows, 0:1], scalar2=mv[:rows, 1:2],
                    op0=ALU.subtract, op1=ALU.mult)
                if cfg.apply_gb1:
                    nc.vector.tensor_tensor(
                        out=xn_sb[:rows, bt, :], in0=xn_sb[:rows, bt, :],
                        in1=gb_sb[:, 0, :].partition_broadcast(rows),
                        op=ALU.mult)
                    nc.vector.tensor_tensor(
                        out=xn_sb[:rows, bt, :], in0=xn_sb[:rows, bt, :],
                        in1=gb_sb[:, 1, :].partition_broadcast(rows),
                        op=ALU.add)
                for ci in range(2):
                    pt = ppool.tile([P, P], BF16, tag="tr")
                    nc.tensor.transpose(
                        out=pt[:], in_=xn_sb[:, bt, ci * P:(ci + 1) * P],
                        identity=ident_b[:])
                    nc.scalar.copy(out=xnt_sb[:, ci, bt, :], in_=pt[:])


def _stage2_kv(nc, tc, cfg, xnt_sb, wqkv_sb, brow_sb, kv_loc, b):
    """K,V projections -> fp8 rows [K_b | V_b] for one batch."""
    d, rt, nloc = D, cfg.rt, cfg.nloc
    with tc.tile_pool(name=f"s2kv_{b}", bufs=4) as pool, \
         tc.tile_pool(name=f"s2kvp_{b}", bufs=2, space="PSUM") as ppool:
        if True:
            for irt in range(rt):
                bt = b * rt + irt
                rows = min(P, nloc - irt * P)
                ps = ppool.tile([P, 2 * d], F32, tag="kv")
                for ci in range(2):
                    nc.tensor.matmul(
                        out=ps[:], lhsT=xnt_sb[:, ci, bt, :],
                        rhs=wqkv_sb[:, ci, d:3 * d],
                        start=(ci == 0), stop=(ci == 1))
                kvb = pool.tile([P, 2 * d], FP8, tag="kvb")
                if cfg.apply_bqkv:
                    nc.vector.tensor_tensor(
                        out=kvb[:rows], in0=ps[:rows],
                        in1=brow_sb[:, d:3 * d].partition_broadcast(rows),
                        op=ALU.add)
                else:
                    nc.scalar.copy(out=kvb[:rows], in_=ps[:rows])
                nc.sync.dma_start(
                    out=kv_loc[irt * P: irt * P + rows, :],
                    in_=kvb[:rows])


def _stage2_q(nc, tc, cfg, xnt_sb, wqkv_sb, brow_sb, q_sb):
    """Q rows (pre-scaled by 1/sqrt(dh) via host-scaled wq) in bf16."""
    d, rt, nloc = D, cfg.rt, cfg.nloc
    with tc.tile_pool(name="s2q", bufs=3) as pool, \
         tc.tile_pool(name="s2qp", bufs=2, space="PSUM") as ppool:
        for b in range(cfg.b):
            for irt in range(rt):
                bt = b * rt + irt
                rows = min(P, nloc - irt * P)
                ps = ppool.tile([P, d], F32, tag="q")
                for ci in range(2):
                    nc.tensor.matmul(
                        out=ps[:], lhsT=xnt_sb[:, ci, bt, :],
                        rhs=wqkv_sb[:, ci, 0:d],
                        start=(ci == 0), stop=(ci == 1))
                if rows < P:
                    nc.vector.memset(q_sb[:, b, irt, :], 0.0)
                if cfg.apply_bqkv:
                    nc.vector.tensor_tensor(
                        out=q_sb[:rows, b, irt, :], in0=ps[:rows],
                        in1=brow_sb[:, 0:d].partition_broadcast(rows),
                        op=ALU.add)
                else:
                    nc.scalar.copy(out=q_sb[:rows, b, irt, :], in_=ps[:rows])


def _stage4(nc, tc, cfg, idx_sb, kv_full, q_sb, xn_sb, sseg_sb, m8_sb,
            mask_sb, wo_sb, ident_b, eps_sb, gb_sb, brow_sb, out_ext, b):
    d, rt, nloc = D, cfg.rt, cfg.nloc
    npt, tpb, nblk = cfg.npt, cfg.tpb, cfg.nblk
    row_elems = 2 * d  # fp8 elements per kv row
    cpt = P // 16              # idx columns per 128-edge tile
    spb = tpb // SUBT          # sub-gathers per block
    first_pad_tile = 0 if cfg.mask_all else \
        ((nloc // npt) if nloc % npt else cfg.nt)

    with tc.tile_pool(name=f"s4g_{b}", bufs=3) as gpool, \
         tc.tile_pool(name=f"s4t_{b}", bufs=3) as tpool, \
         tc.tile_pool(name=f"s4pq_{b}", bufs=1, space="PSUM") as pqpool, \
         tc.tile_pool(name=f"s4pb_{b}", bufs=2, space="PSUM") as pbpool:
        for blk in range(nblk):
            buf = gpool.tile([P, spb, SUBT, row_elems], FP8, tag="rows")
            for j in range(spb):
                t0 = blk * tpb + j * SUBT
                nc.gpsimd.dma_gather(
                    out_ap=buf[:, j, :, :],
                    in_ap=kv_full[:, :],
                    idxs_ap=idx_sb[:, t0 * cpt: (t0 + SUBT) * cpt],
                    num_idxs=SUBT * P, num_idxs_reg=SUBT * P,
                    elem_size=row_elems, elem_step=row_elems,
                    queue_num=(blk * spb + j) % cfg.swdge_queues)
            bufb = tpool.tile([P, spb, SUBT, row_elems], BF16, tag="rowsb")
            nc.vector.tensor_copy(out=bufb[:], in_=buf[:])
            nb = pbpool.tile([P, d + 8], F32, tag="nb")
            qrep = pqpool.tile([P, tpb, d], F32, tag="qrep")
            for sl in range(tpb):
                nc.tensor.matmul(
                    out=qrep[:, sl, :], lhsT=m8_sb[:, sl, :],
                    rhs=q_sb[:, b, blk, :],
                    start=True, stop=True)
            qrep_sb = tpool.tile([P, tpb, d], BF16, tag="qrepsb")
            nc.scalar.copy(out=qrep_sb[:], in_=qrep[:])
            qk = tpool.tile([P, tpb, d], BF16, tag="qk")
            nc.vector.tensor_tensor(
                out=qk[:].rearrange("p (j t) q -> p j t q", j=spb),
                in0=qrep_sb[:].rearrange("p (j t) q -> p j t q", j=spb),
                in1=bufb[:, :, :, 0:d],
                op=ALU.mult)
            qkh = tpool.tile([P, tpb, 8, DH // 2], BF16, tag="qkh")
            nc.vector.tensor_tensor(
                out=qkh[:],
                in0=qk[:].rearrange("p t (h x) -> p t h x", x=DH)[:, :, :, 0:DH // 2],
                in1=qk[:].rearrange("p t (h x) -> p t h x", x=DH)[:, :, :, DH // 2:DH],
                op=ALU.add)
            lg = tpool.tile([P, tpb, 8], F32, tag="lg")
            nc.vector.tensor_reduce(
                out=lg[:], in_=qkh[:],
                axis=mybir.AxisListType.X, op=ALU.add)
            rhs_t = tpool.tile([P, tpb, d + 8], BF16, tag="rhs")
            nc.scalar.activation(
                out=rhs_t[:, :, d:d + 8], in_=lg[:], func=AF.Exp,
                scale=1.0)
            t_first = blk * tpb
            if t_first + tpb > first_pad_tile:
                nc.vector.tensor_tensor(
                    out=rhs_t[:, :, d:d + 8],
                    in0=rhs_t[:, :, d:d + 8],
                    in1=mask_sb[:, t_first:t_first + tpb]
                    .unsqueeze(-1).broadcast_to([P, tpb, 8]),
                    op=ALU.mult)
            nc.vector.tensor_tensor(
                out=rhs_t[:, :, :d].rearrange("p t (h x) -> p t h x", x=DH),
                in0=bufb[:, :, :, d:2 * d]
                .rearrange("p j t q -> p (j t) q")
                .rearrange("p t (h x) -> p t h x", x=DH),
                in1=rhs_t[:, :, d:d + 8]
                .unsqueeze(-1).broadcast_to([P, tpb, 8, DH]),
                op=ALU.mult)
            for sl in range(tpb):
                nc.tensor.matmul(
                    out=nb[:], lhsT=sseg_sb[:, sl, :],
                    rhs=rhs_t[:, sl, :],
                    start=(sl == 0), stop=(sl == tpb - 1))
            base = blk * P
            valid = min(P, nloc - base)
            bt = b * rt + blk
            rec = tpool.tile([P, 8], F32, tag="rec")
            nc.vector.reciprocal(out=rec[:valid], in_=nb[:valid, d:d + 8])
            tmp = tpool.tile([P, d], BF16, tag="attn")
            nc.vector.tensor_tensor(
                out=tmp[:valid].rearrange("p (h x) -> p h x", x=DH),
                in0=nb[:valid, :d].rearrange("p (h x) -> p h x", x=DH),
                in1=rec[:valid].unsqueeze(-1).broadcast_to([valid, H, DH]),
                op=ALU.mult)
            nc.vector.tensor_tensor(
                out=xn_sb[:valid, bt, :], in0=xn_sb[:valid, bt, :],
                in1=tmp[:valid], op=ALU.add)


def _stage5(nc, tc, cfg, xn_sb, wo_sb, ident_b, eps_sb, gb_sb, brow_sb,
            out_ext, b):
    with tc.tile_pool(name=f"s5_{b}", bufs=4) as pool, \
         tc.tile_pool(name=f"s5o_{b}", bufs=3) as opool, \
         tc.tile_pool(name=f"s5p_{b}", bufs=2, space="PSUM") as ppool:
        for blk in range(cfg.nblk):
            valid = min(P, cfg.nloc - blk * P)
            _stage5_block(nc, cfg, pool, opool, ppool, xn_sb, wo_sb, ident_b,
                          eps_sb, gb_sb, brow_sb, out_ext, b, blk, valid)


def _stage5_block(nc, cfg, pool, opool, ppool, xn_sb, wo_sb, ident_b, eps_sb,
                  gb_sb, brow_sb, out_ext, b, blk, rows):
    """Fused output stage for one 128-row block (concat rows are final)."""
    d, rt, nloc = D, cfg.rt, cfg.nloc
    bt = b * rt + blk
    mv = _layer_norm_rs(nc, pool, xn_sb[:rows, bt, :], rows, eps_sb)
    cnb = pool.tile([P, d], BF16, tag="cnb")
    if rows < P:
        nc.vector.memset(cnb[:], 0.0)
    if cfg.apply_gb2:
        cn32 = pool.tile([P, d], F32, tag="cn32")
        nc.vector.tensor_scalar(
            out=cn32[:rows], in0=xn_sb[:rows, bt, :],
            scalar1=mv[:rows, 0:1], scalar2=mv[:rows, 1:2],
            op0=ALU.subtract, op1=ALU.mult)
        nc.vector.tensor_tensor(
            out=cn32[:rows], in0=cn32[:rows],
            in1=gb_sb[:, 2, :].partition_broadcast(rows), op=ALU.mult)
        nc.vector.tensor_tensor(
            out=cnb[:rows], in0=cn32[:rows],
            in1=gb_sb[:, 3, :].partition_broadcast(rows), op=ALU.add)
    else:
        nc.vector.tensor_scalar(
            out=cnb[:rows], in0=xn_sb[:rows, bt, :],
            scalar1=mv[:rows, 0:1], scalar2=mv[:rows, 1:2],
            op0=ALU.subtract, op1=ALU.mult)
    y = ppool.tile([P, d], F32, tag="y")
    for ci in range(2):
        pt = ppool.tile([P, P], BF16, tag="tr5")
        nc.tensor.transpose(out=pt[:], in_=cnb[:, ci * P:(ci + 1) * P],
                            identity=ident_b[:])
        cnt = pool.tile([P, P], BF16, tag="cnt")
        nc.scalar.copy(out=cnt[:], in_=pt[:])
        nc.tensor.matmul(out=y[:], lhsT=cnt[:], rhs=wo_sb[:, ci, :],
                         start=(ci == 0), stop=(ci == 1))
    if cfg.apply_bo:
        nc.vector.tensor_tensor(
            out=y[:rows], in0=y[:rows],
            in1=brow_sb[:, 3 * d: 4 * d].partition_broadcast(rows),
            op=ALU.add)
    ot = opool.tile([P, d], F32, tag="ot")
    nc.vector.scalar_tensor_tensor(
        out=ot[:rows], in0=y[:rows], scalar=0.0, in1=xn_sb[:rows, bt, :],
        op0=ALU.max, op1=ALU.add)
    nc.sync.dma_start(
        out=out_ext[b * nloc + blk * P: b * nloc + blk * P + rows, :],
        in_=ot[:rows])


# ------------------------------------------------------------------ host side
def _prep_edges(edges: np.ndarray, cfg: Cfg):
    """Group edges by src, pad each node to cfg.r slots.

    Returns per-core lists: idx [128, nt_pad*8] int16, mask [128, nt_pad] f32.
    """
    n, nloc, r, nt_pad = cfg.n, cfg.nloc, cfg.r, cfg.nt_pad
    src = np.asarray(edges[:, 0], dtype=np.int64)
    dst = np.asarray(edges[:, 1], dtype=np.int64)
    order = np.argsort(src, kind="stable")
    src_s, dst_s = src[order], dst[order]
    counts = np.bincount(src_s, minlength=n)
    assert counts.max() <= r, f"node degree {counts.max()} > r={r}"
    starts = np.zeros(n, dtype=np.int64)
    starts[1:] = np.cumsum(counts)[:-1]
    slot = (np.arange(len(src_s)) - starts[src_s]) + src_s * r
    dst_pad = np.zeros(n * r, dtype=np.int16)
    mask_pad = np.zeros(n * r, dtype=np.float32)
    dst_pad[slot] = dst_s.astype(np.int16)
    mask_pad[slot] = 1.0
    idx_list, mask_list = [], []
    pad_to = nt_pad * P
    for c in range(cfg.ncores):
        dp = np.pad(dst_pad[c * nloc * r:(c + 1) * nloc * r],
                    (0, pad_to - nloc * r))
        mp = np.pad(mask_pad[c * nloc * r:(c + 1) * nloc * r],
                    (0, pad_to - nloc * r))
        # flat position j = T*128 + p, p = node_in_tile*r + slot
        idx_w = np.tile(dp.reshape(-1, 16).T, (8, 1))  # [128, nt_pad*8]
        mtiles = mp.reshape(nt_pad, P).T.copy()        # [128, nt_pad]
        idx_list.append(np.ascontiguousarray(idx_w, dtype=np.int16))
        mask_list.append(np.ascontiguousarray(mtiles, dtype=np.float32))
    return idx_list, mask_list, counts


def _to_bf16(a):
    import ml_dtypes
    return np.asarray(a, dtype=np.float32).astype(ml_dtypes.bfloat16)


def _seg_mats(cfg: Cfg):
    p = np.arange(P)
    tpb = cfg.tpb
    sseg = np.zeros((P, tpb, P), np.float32)
    for k in range(tpb):
        sseg[p, k, cfg.npt * k + p // cfg.r] = 1.0
    sseg = sseg.reshape(P, tpb * P)
    m8 = np.zeros((P, 8, P), np.float32)
    for k in range(8):
        m8[np.arange(P), k, :] = 0.0
        for e in range(P):
            m8[k * 16 + e // cfg.r, k, e] = 1.0
    m8 = m8.reshape(P, 8 * P)
    return sseg, m8


_PROG_CACHE: dict = {}


def get_program(cfg: Cfg):
    if cfg not in _PROG_CACHE:
        _PROG_CACHE[cfg] = build_program(cfg)
    return _PROG_CACHE[cfg]


def make_cfg(inputs, **overrides) -> Cfg:
    gamma1 = np.asarray(inputs["gamma1"], np.float32)
    beta1 = np.asarray(inputs["beta1"], np.float32)
    gamma2 = np.asarray(inputs["gamma2"], np.float32)
    beta2 = np.asarray(inputs["beta2"], np.float32)
    bqkv = np.concatenate([np.asarray(inputs["bq"], np.float32),
                           np.asarray(inputs["bk"], np.float32),
                           np.asarray(inputs["bv"], np.float32)])
    bo = np.asarray(inputs["bo"], np.float32)
    edges = np.asarray(inputs["edges"])
    n = overrides.get("n", N)
    counts = np.bincount(np.asarray(edges[:, 0], np.int64), minlength=n)
    r = 8
    while r < counts.max():
        r *= 2
    assert r <= P
    return Cfg(
        r=r,
        mask_all=bool(counts.min() < r),
        apply_gb1=not (np.all(gamma1 == 1) and np.all(beta1 == 0)),
        apply_gb2=not (np.all(gamma2 == 1) and np.all(beta2 == 0)),
        apply_bqkv=bool(np.any(bqkv != 0)),
        apply_bo=bool(np.any(bo != 0)),
        **overrides,
    )


def make_in_maps(inputs: dict, cfg: Cfg):
    import ml_dtypes
    x = np.asarray(inputs["x"], dtype=np.float32)
    edges = np.asarray(inputs["edges"])
    s = 1.0 / math.sqrt(DH)
    wqkv = np.concatenate([np.asarray(inputs["wq"], np.float32) * s,
                           np.asarray(inputs["wk"], np.float32),
                           np.asarray(inputs["wv"], np.float32)], axis=1)
    wo = np.asarray(inputs["wo"], np.float32)
    bqkv = np.concatenate([np.asarray(inputs["bq"], np.float32) * s,
                           np.asarray(inputs["bk"], np.float32),
                           np.asarray(inputs["bv"], np.float32)])
    bo = np.asarray(inputs["bo"], np.float32)
    gb = np.stack([np.asarray(inputs["gamma1"], np.float32),
                   np.asarray(inputs["beta1"], np.float32),
                   np.asarray(inputs["gamma2"], np.float32),
                   np.asarray(inputs["beta2"], np.float32)])

    idx_list, mask_list, _ = _prep_edges(edges, cfg)
    sseg, m8 = _seg_mats(cfg)
    brow = np.concatenate([bqkv, bo])[None, :].astype(np.float32)

    wqkv_b = _to_bf16(wqkv)
    wo_b = _to_bf16(wo)
    sseg_b = _to_bf16(sseg)
    m8_b = _to_bf16(m8)
    in_maps = []
    for c in range(cfg.ncores):
        lo, hi = c * cfg.nloc, (c + 1) * cfg.nloc
        x_loc = np.ascontiguousarray(x[:, lo:hi, :].reshape(cfg.b * cfg.nloc, D))
        in_maps.append({
            "x": x_loc,
            "wqkv": wqkv_b,
            "wo": wo_b,
            "idx": idx_list[c],
            "mask": mask_list[c],
            "sseg": sseg_b,
            "m8": m8_b,
            "gb": gb,
            "brow": brow,
        })
    return in_maps


def assemble_out(results, cfg: Cfg):
    out = np.empty((cfg.b, cfg.n, D), dtype=np.float32)
    for c in range(cfg.ncores):
        o = np.asarray(results[c]["out"]).reshape(cfg.b, cfg.nloc, D)
        out[:, c * cfg.nloc:(c + 1) * cfg.nloc, :] = o
    return out


LAST_RESULT = None  # BassKernelResults of the most recent kernel() call


def kernel(**inputs) -> np.ndarray:
    global LAST_RESULT
    from concourse.bass_utils import run_bass_kernel_spmd

    cfg = make_cfg(inputs)
    nc = get_program(cfg)
    in_maps = make_in_maps(inputs, cfg)
    LAST_RESULT = run_bass_kernel_spmd(nc, in_maps, list(range(cfg.ncores)))
    return assemble_out(LAST_RESULT.results, cfg)


# revision 21
# speedup vs baseline: 1.1161x; 1.1161x over previous
"""Sparse (graph-edge) multi-head attention block on 8 TRN2 NeuronCores.

Problem: nn_MultiHeadAttention_6966436954266
  B=2, N=20000, D=256, H=8, dh=32, E=160000 (8 out-edges per node, sorted by src)

  xn  = LN1(x); q,k,v = xn @ w{q,k,v}; per-edge w = exp(q_src.k_dst/sqrt(dh))
  attn = segment_sum(w*v_dst)/segment_sum(w); concat = xn + attn
  out = relu(LN2(concat) @ wo + bo) + concat

Sharding: nodes partitioned contiguously across 8 cores (2500 each). Each core
LN+projects its shard into an fp8 K/V row table (row = [K0 V0 K1 V1], 1KB),
AllGathered into one Shared 20MB table; each core then runs the edge stage for
its own nodes' edges (grouped by src, 8 slots/node).

Edge-stage per 128-node block (8 tiles of 16 nodes x 8 slots):
  - ONE dma_gather per 512 edge slots fetches the full 1KB row (K and V for
    both batches) -> [128, 4, 1024] fp8. SWDGE descriptor emission (~8.6ns per
    descriptor, measured) is the gather bottleneck, so descriptor count is
    minimized by fetching everything an edge needs in one descriptor.
  - q rows are broadcast node->edge-slots with a constant [16,128] selector
    matmul on PE (out = M16^T @ q_tile), PSUM f32.
  - qk product on DVE (fp8 K upconverts in-op), per-head logits via strided
    tensor_reduce, exp on ACT (wq/bq are pre-scaled by 1/sqrt(dh) on host).
  - w*v on DVE with free-dim broadcast; segment sum via PE (lhsT = constant
    0/1 slot-selector), accumulating [128 nodes, 256+8] in PSUM per block.
  - Block epilogue (attn=num/den, concat=xn+attn, LN2 + wo matmul + relu +
    residual + store) runs fused, overlapping later blocks' gathers.
"""

import math
from dataclasses import dataclass

import numpy as np

import concourse.bass as bass
import concourse.bacc as bacc
import concourse.mybir as mybir
import concourse.tile as tile
from concourse.masks import make_identity

B, N, D, H, DH = 2, 20000, 256, 8, 32
NCORES = 8
EPS = 1e-3
P = 128
F32 = mybir.dt.float32
BF16 = mybir.dt.bfloat16
FP8 = mybir.dt.float8e4
I16 = mybir.dt.int16
AF = mybir.ActivationFunctionType
ALU = mybir.AluOpType
SUBT = 4  # tiles per dma_gather (512-index SWDGE ring limit)


@dataclass(frozen=True)
class Cfg:
    n: int = N
    nloc: int = N // NCORES
    ncores: int = NCORES
    b: int = B
    r: int = 8          # edge slots per node (pow2, divides 128)
    mask_all: bool = False
    apply_gb1: bool = False
    apply_gb2: bool = False
    apply_bqkv: bool = False
    apply_bo: bool = False
    swdge_queues: int = 4

    @property
    def npt(self):  # nodes per 128-edge tile
        return P // self.r

    @property
    def nt(self):  # real edge tiles per batch
        return math.ceil(self.nloc / self.npt)

    @property
    def tpb(self):  # tiles per 128-node block
        return P // self.npt

    @property
    def nblk(self):  # 128-node blocks (gather/segment granularity)
        return math.ceil(self.nt / self.tpb)

    @property
    def nt_pad(self):  # idx tiles padded to whole blocks
        return self.nblk * self.tpb

    @property
    def rt(self):  # 128-row tiles per batch (dense stages)
        return math.ceil(self.nloc / P)


def _ceil_div(a, b):
    return (a + b - 1) // b


# ------------------------------------------------------------------- program
def build_program(cfg: Cfg) -> bass.Bass:
    nc = bacc.Bacc("TRN2", num_devices=cfg.ncores,
                   num_swdge_queues=cfg.swdge_queues)
    d = D
    rt = cfg.rt
    nloc, b_ = cfg.nloc, cfg.b

    x_in = nc.dram_tensor("x", [b_ * nloc, d], F32, kind="ExternalInput")
    wqkv_in = nc.dram_tensor("wqkv", [d, 3 * d], BF16, kind="ExternalInput")
    wo_in = nc.dram_tensor("wo", [d, d], BF16, kind="ExternalInput")
    idx_in = nc.dram_tensor("idx", [P, cfg.nt_pad * (P // 16)], I16,
                            kind="ExternalInput")
    mask_in = nc.dram_tensor("mask", [P, cfg.nt_pad], F32, kind="ExternalInput")
    sseg_in = nc.dram_tensor("sseg", [P, cfg.tpb * P], BF16,
                             kind="ExternalInput")
    m8_in = nc.dram_tensor("m8", [P, 8 * P], BF16, kind="ExternalInput")
    gb_in = nc.dram_tensor("gb", [4, d], F32, kind="ExternalInput")
    brow_in = nc.dram_tensor("brow", [1, 4 * d], F32, kind="ExternalInput")
    out_ext = nc.dram_tensor("out", [b_ * nloc, d], F32, kind="ExternalOutput")

    kv_loc = [nc.dram_tensor(f"kv_loc{b}", [nloc, 2 * d], BF16)
              for b in range(b_)]
    kv_full = [nc.dram_tensor(
        f"kv_full{b}", [cfg.n, 2 * d], BF16,
        addr_space="Shared" if cfg.ncores > 4 else "Local")
        for b in range(b_)]

    with tile.TileContext(nc) as tc:
        with tc.tile_pool(name="const", bufs=1) as cpool:
            ident_f = cpool.tile([P, P], F32)
            make_identity(nc, ident_f[:])
            ident_b = cpool.tile([P, P], BF16)
            make_identity(nc, ident_b[:])
            wqkv_sb = cpool.tile([P, 2, 3 * d], BF16)
            nc.sync.dma_start(
                out=wqkv_sb[:],
                in_=wqkv_in[:].rearrange("(ci p) q -> p ci q", p=P))
            wo_sb = cpool.tile([P, 2, d], BF16)
            nc.sync.dma_start(
                out=wo_sb[:], in_=wo_in[:].rearrange("(ci p) q -> p ci q", p=P))
            sseg_sb = cpool.tile([P, cfg.tpb, P], BF16)
            nc.sync.dma_start(
                out=sseg_sb[:],
                in_=sseg_in[:].rearrange("p (k m) -> p k m", k=cfg.tpb))
            m8_sb = cpool.tile([P, 8, P], BF16)
            nc.sync.dma_start(
                out=m8_sb[:], in_=m8_in[:].rearrange("p (k m) -> p k m", k=8))
            idx_sb = cpool.tile([P, cfg.nt_pad * (P // 16)], I16)
            nc.sync.dma_start(out=idx_sb[:], in_=idx_in[:])
            mask_sb = cpool.tile([P, cfg.nt_pad], F32)
            if cfg.mask_all or cfg.nloc % cfg.npt:
                nc.sync.dma_start(out=mask_sb[:], in_=mask_in[:])
            eps_sb = cpool.tile([P, 1], F32)
            nc.vector.memset(eps_sb[:], EPS)
            gb_sb = cpool.tile([1, 4, d], F32)
            if cfg.apply_gb1 or cfg.apply_gb2:
                nc.sync.dma_start(out=gb_sb[:],
                                  in_=gb_in[:].rearrange("g d -> 1 g d"))
            brow_sb = cpool.tile([1, 4 * d], F32)
            if cfg.apply_bqkv or cfg.apply_bo:
                nc.sync.dma_start(out=brow_sb[:], in_=brow_in[:])

            with tc.tile_pool(name="resident", bufs=1) as rpool:
                xn_sb = rpool.tile([P, b_ * rt, d], BF16)
                xnt_sb = rpool.tile([P, 2, b_ * rt, P], BF16)
                q_sb = rpool.tile([P, b_, rt, d], BF16)
                for b in range(b_):
                    _stage1(nc, tc, cfg, x_in, xn_sb, xnt_sb, ident_b, eps_sb,
                            gb_sb, b)
                    _stage2_kv(nc, tc, cfg, xnt_sb, wqkv_sb, brow_sb,
                               kv_loc[b], b)
                    nc.gpsimd.collective_compute(
                        "AllGather",
                        ALU.bypass,
                        replica_groups=[list(range(cfg.ncores))],
                        ins=[kv_loc[b][:]],
                        outs=[kv_full[b][:]],
                    )
                _stage2_q(nc, tc, cfg, xnt_sb, wqkv_sb, brow_sb, q_sb)
                for b in range(b_):
                    _stage4(nc, tc, cfg, idx_sb, kv_full[b], q_sb, xn_sb,
                            sseg_sb, m8_sb, mask_sb, wo_sb, ident_b, eps_sb,
                            gb_sb, brow_sb, out_ext, b)
                    _stage5(nc, tc, cfg, xn_sb, wo_sb, ident_b, eps_sb, gb_sb,
                            brow_sb, out_ext, b)
    nc.finalize()
    return nc


def _layer_norm_rs(nc, pool, src_ap, rows, eps_sb):
    """bn_stats -> mv [P,2] f32 with [:,0]=mean, [:,1]=1/sqrt(var+eps)."""
    stats = pool.tile([P, 6], F32, tag="ln_stats")
    nc.vector.bn_stats(out=stats[:rows], in_=src_ap)
    mv = pool.tile([P, 2], F32, tag="ln_mv")
    nc.vector.bn_aggr(out=mv[:rows], in_=stats[:rows])
    nc.scalar.activation(out=mv[:rows, 1:2], in_=mv[:rows, 1:2], func=AF.Sqrt,
                         bias=eps_sb[:rows], scale=1.0)
    nc.vector.reciprocal(out=mv[:rows, 1:2], in_=mv[:rows, 1:2])
    return mv


def _stage1(nc, tc, cfg, x_in, xn_sb, xnt_sb, ident_b, eps_sb, gb_sb, b):
    d, rt, nloc = D, cfg.rt, cfg.nloc
    with tc.tile_pool(name=f"s1_{b}", bufs=6) as pool, \
         tc.tile_pool(name=f"s1p_{b}", bufs=6, space="PSUM") as ppool:
        if True:
            for irt in range(rt):
                bt = b * rt + irt
                rows = min(P, nloc - irt * P)
                xt = pool.tile([P, d], F32, tag="xt")
                nc.sync.dma_start(
                    out=xt[:rows],
                    in_=x_in[b * nloc + irt * P: b * nloc + irt * P + rows, :])
                if rows < P:
                    nc.vector.memset(xn_sb[:, bt, :], 0.0)
                mv = _layer_norm_rs(nc, pool, xt[:rows], rows, eps_sb)
                nc.vector.tensor_scalar(
                    out=xn_sb[:rows, bt, :], in0=xt[:rows],
                    scalar1=mv[:rows, 0:1], scalar2=mv[:rows, 1:2],
                    op0=ALU.subtract, op1=ALU.mult)
                if cfg.apply_gb1:
                    nc.vector.tensor_tensor(
                        out=xn_sb[:rows, bt, :], in0=xn_sb[:rows, bt, :],
                        in1=gb_sb[:, 0, :].partition_broadcast(rows),
                        op=ALU.mult)
                    nc.vector.tensor_tensor(
                        out=xn_sb[:rows, bt, :], in0=xn_sb[:rows, bt, :],
                        in1=gb_sb[:, 1, :].partition_broadcast(rows),
                        op=ALU.add)
                for ci in range(2):
                    pt = ppool.tile([P, P], BF16, tag="tr")
                    nc.tensor.transpose(
                        out=pt[:], in_=xn_sb[:, bt, ci * P:(ci + 1) * P],
                        identity=ident_b[:])
                    nc.scalar.copy(out=xnt_sb[:, ci, bt, :], in_=pt[:])


def _stage2_kv(nc, tc, cfg, xnt_sb, wqkv_sb, brow_sb, kv_loc, b):
    """K,V projections -> fp8 rows [K_b | V_b] for one batch."""
    d, rt, nloc = D, cfg.rt, cfg.nloc
    with tc.tile_pool(name=f"s2kv_{b}", bufs=4) as pool, \
         tc.tile_pool(name=f"s2kvp_{b}", bufs=2, space="PSUM") as ppool:
        if True:
            for irt in range(rt):
                bt = b * rt + irt
                rows = min(P, nloc - irt * P)
                ps = ppool.tile([P, 2 * d], F32, tag="kv")
                for ci in range(2):
                    nc.tensor.matmul(
                        out=ps[:], lhsT=xnt_sb[:, ci, bt, :],
                        rhs=wqkv_sb[:, ci, d:3 * d],
                        start=(ci == 0), stop=(ci == 1))
                kvb = pool.tile([P, 2 * d], BF16, tag="kvb")
                if cfg.apply_bqkv:
                    nc.vector.tensor_tensor(
                        out=kvb[:rows], in0=ps[:rows],
                        in1=brow_sb[:, d:3 * d].partition_broadcast(rows),
                        op=ALU.add)
                else:
                    nc.scalar.copy(out=kvb[:rows], in_=ps[:rows])
                nc.sync.dma_start(
                    out=kv_loc[irt * P: irt * P + rows, :],
                    in_=kvb[:rows])


def _stage2_q(nc, tc, cfg, xnt_sb, wqkv_sb, brow_sb, q_sb):
    """Q rows (pre-scaled by 1/sqrt(dh) via host-scaled wq) in bf16."""
    d, rt, nloc = D, cfg.rt, cfg.nloc
    with tc.tile_pool(name="s2q", bufs=3) as pool, \
         tc.tile_pool(name="s2qp", bufs=2, space="PSUM") as ppool:
        for b in range(cfg.b):
            for irt in range(rt):
                bt = b * rt + irt
                rows = min(P, nloc - irt * P)
                ps = ppool.tile([P, d], F32, tag="q")
                for ci in range(2):
                    nc.tensor.matmul(
                        out=ps[:], lhsT=xnt_sb[:, ci, bt, :],
                        rhs=wqkv_sb[:, ci, 0:d],
                        start=(ci == 0), stop=(ci == 1))
                if rows < P:
                    nc.vector.memset(q_sb[:, b, irt, :], 0.0)
                if cfg.apply_bqkv:
                    nc.vector.tensor_tensor(
                        out=q_sb[:rows, b, irt, :], in0=ps[:rows],
                        in1=brow_sb[:, 0:d].partition_broadcast(rows),
                        op=ALU.add)
                else:
                    nc.scalar.copy(out=q_sb[:rows, b, irt, :], in_=ps[:rows])


def _stage4(nc, tc, cfg, idx_sb, kv_full, q_sb, xn_sb, sseg_sb, m8_sb,
            mask_sb, wo_sb, ident_b, eps_sb, gb_sb, brow_sb, out_ext, b):
    d, rt, nloc = D, cfg.rt, cfg.nloc
    npt, tpb, nblk = cfg.npt, cfg.tpb, cfg.nblk
    row_elems = 2 * d  # fp8 elements per kv row
    cpt = P // 16              # idx columns per 128-edge tile
    spb = tpb // SUBT          # sub-gathers per block
    first_pad_tile = 0 if cfg.mask_all else \
        ((nloc // npt) if nloc % npt else cfg.nt)

    with tc.tile_pool(name=f"s4g_{b}", bufs=3) as gpool, \
         tc.tile_pool(name=f"s4t_{b}", bufs=3) as tpool, \
         tc.tile_pool(name=f"s4pq_{b}", bufs=1, space="PSUM") as pqpool, \
         tc.tile_pool(name=f"s4pb_{b}", bufs=2, space="PSUM") as pbpool:
        for blk in range(nblk):
            buf = gpool.tile([P, spb, SUBT, row_elems], BF16, tag="rows")
            for j in range(spb):
                t0 = blk * tpb + j * SUBT
                nc.gpsimd.dma_gather(
                    out_ap=buf[:, j, :, :],
                    in_ap=kv_full[:, :],
                    idxs_ap=idx_sb[:, t0 * cpt: (t0 + SUBT) * cpt],
                    num_idxs=SUBT * P, num_idxs_reg=SUBT * P,
                    elem_size=row_elems, elem_step=row_elems,
                    queue_num=(blk * spb + j) % cfg.swdge_queues)
            nb = pbpool.tile([P, d + 8], F32, tag="nb")
            qrep = pqpool.tile([P, tpb, d], F32, tag="qrep")
            for sl in range(tpb):
                nc.tensor.matmul(
                    out=qrep[:, sl, :], lhsT=m8_sb[:, sl, :],
                    rhs=q_sb[:, b, blk, :],
                    start=True, stop=True)
            qrep_sb = tpool.tile([P, tpb, d], BF16, tag="qrepsb")
            nc.scalar.copy(out=qrep_sb[:], in_=qrep[:])
            qk = tpool.tile([P, tpb, d], BF16, tag="qk")
            nc.vector.tensor_tensor(
                out=qk[:].rearrange("p (j t) q -> p j t q", j=spb),
                in0=qrep_sb[:].rearrange("p (j t) q -> p j t q", j=spb),
                in1=buf[:, :, :, 0:d],
                op=ALU.mult)
            qkh = tpool.tile([P, tpb, 8, DH // 2], BF16, tag="qkh")
            nc.vector.tensor_tensor(
                out=qkh[:],
                in0=qk[:].rearrange("p t (h x) -> p t h x", x=DH)[:, :, :, 0:DH // 2],
                in1=qk[:].rearrange("p t (h x) -> p t h x", x=DH)[:, :, :, DH // 2:DH],
                op=ALU.add)
            lg = tpool.tile([P, tpb, 8], F32, tag="lg")
            nc.vector.tensor_reduce(
                out=lg[:], in_=qkh[:],
                axis=mybir.AxisListType.X, op=ALU.add)
            rhs_t = tpool.tile([P, tpb, d + 8], BF16, tag="rhs")
            nc.scalar.activation(
                out=rhs_t[:, :, d:d + 8], in_=lg[:], func=AF.Exp,
                scale=1.0)
            t_first = blk * tpb
            if t_first + tpb > first_pad_tile:
                nc.vector.tensor_tensor(
                    out=rhs_t[:, :, d:d + 8],
                    in0=rhs_t[:, :, d:d + 8],
                    in1=mask_sb[:, t_first:t_first + tpb]
                    .unsqueeze(-1).broadcast_to([P, tpb, 8]),
                    op=ALU.mult)
            nc.vector.tensor_tensor(
                out=rhs_t[:, :, :d].rearrange("p t (h x) -> p t h x", x=DH),
                in0=buf[:, :, :, d:2 * d]
                .rearrange("p j t q -> p (j t) q")
                .rearrange("p t (h x) -> p t h x", x=DH),
                in1=rhs_t[:, :, d:d + 8]
                .unsqueeze(-1).broadcast_to([P, tpb, 8, DH]),
                op=ALU.mult)
            for sl in range(tpb):
                nc.tensor.matmul(
                    out=nb[:], lhsT=sseg_sb[:, sl, :],
                    rhs=rhs_t[:, sl, :],
                    start=(sl == 0), stop=(sl == tpb - 1))
            base = blk * P
            valid = min(P, nloc - base)
            bt = b * rt + blk
            rec = tpool.tile([P, 8], F32, tag="rec")
            nc.vector.reciprocal(out=rec[:valid], in_=nb[:valid, d:d + 8])
            tmp = tpool.tile([P, d], BF16, tag="attn")
            nc.vector.tensor_tensor(
                out=tmp[:valid].rearrange("p (h x) -> p h x", x=DH),
                in0=nb[:valid, :d].rearrange("p (h x) -> p h x", x=DH),
                in1=rec[:valid].unsqueeze(-1).broadcast_to([valid, H, DH]),
                op=ALU.mult)
            nc.vector.tensor_tensor(
                out=xn_sb[:valid, bt, :], in0=xn_sb[:valid, bt, :],
                in1=tmp[:valid], op=ALU.add)


def _stage5(nc, tc, cfg, xn_sb, wo_sb, ident_b, eps_sb, gb_sb, brow_sb,
            out_ext, b):
    with tc.tile_pool(name=f"s5_{b}", bufs=4) as pool, \
         tc.tile_pool(name=f"s5o_{b}", bufs=3) as opool, \
         tc.tile_pool(name=f"s5p_{b}", bufs=2, space="PSUM") as ppool:
        for blk in range(cfg.nblk):
            valid = min(P, cfg.nloc - blk * P)
            _stage5_block(nc, cfg, pool, opool, ppool, xn_sb, wo_sb, ident_b,
                          eps_sb, gb_sb, brow_sb, out_ext, b, blk, valid)


def _stage5_block(nc, cfg, pool, opool, ppool, xn_sb, wo_sb, ident_b, eps_sb,
                  gb_sb, brow_sb, out_ext, b, blk, rows):
    """Fused output stage for one 128-row block (concat rows are final)."""
    d, rt, nloc = D, cfg.rt, cfg.nloc
    bt = b * rt + blk
    mv = _layer_norm_rs(nc, pool, xn_sb[:rows, bt, :], rows, eps_sb)
    cnb = pool.tile([P, d], BF16, tag="cnb")
    if rows < P:
        nc.vector.memset(cnb[:], 0.0)
    if cfg.apply_gb2:
        cn32 = pool.tile([P, d], F32, tag="cn32")
        nc.vector.tensor_scalar(
            out=cn32[:rows], in0=xn_sb[:rows, bt, :],
            scalar1=mv[:rows, 0:1], scalar2=mv[:rows, 1:2],
            op0=ALU.subtract, op1=ALU.mult)
        nc.vector.tensor_tensor(
            out=cn32[:rows], in0=cn32[:rows],
            in1=gb_sb[:, 2, :].partition_broadcast(rows), op=ALU.mult)
        nc.vector.tensor_tensor(
            out=cnb[:rows], in0=cn32[:rows],
            in1=gb_sb[:, 3, :].partition_broadcast(rows), op=ALU.add)
    else:
        nc.vector.tensor_scalar(
            out=cnb[:rows], in0=xn_sb[:rows, bt, :],
            scalar1=mv[:rows, 0:1], scalar2=mv[:rows, 1:2],
            op0=ALU.subtract, op1=ALU.mult)
    y = ppool.tile([P, d], F32, tag="y")
    for ci in range(2):
        pt = ppool.tile([P, P], BF16, tag="tr5")
        nc.tensor.transpose(out=pt[:], in_=cnb[:, ci * P:(ci + 1) * P],
                            identity=ident_b[:])
        cnt = pool.tile([P, P], BF16, tag="cnt")
        nc.scalar.copy(out=cnt[:], in_=pt[:])
        nc.tensor.matmul(out=y[:], lhsT=cnt[:], rhs=wo_sb[:, ci, :],
                         start=(ci == 0), stop=(ci == 1))
    if cfg.apply_bo:
        nc.vector.tensor_tensor(
            out=y[:rows], in0=y[:rows],
            in1=brow_sb[:, 3 * d: 4 * d].partition_broadcast(rows),
            op=ALU.add)
    ot = opool.tile([P, d], F32, tag="ot")
    nc.vector.scalar_tensor_tensor(
        out=ot[:rows], in0=y[:rows], scalar=0.0, in1=xn_sb[:rows, bt, :],
        op0=ALU.max, op1=ALU.add)
    nc.sync.dma_start(
        out=out_ext[b * nloc + blk * P: b * nloc + blk * P + rows, :],
        in_=ot[:rows])


# ------------------------------------------------------------------ host side
def _prep_edges(edges: np.ndarray, cfg: Cfg):
    """Group edges by src, pad each node to cfg.r slots.

    Returns per-core lists: idx [128, nt_pad*8] int16, mask [128, nt_pad] f32.
    """
    n, nloc, r, nt_pad = cfg.n, cfg.nloc, cfg.r, cfg.nt_pad
    src = np.asarray(edges[:, 0], dtype=np.int64)
    dst = np.asarray(edges[:, 1], dtype=np.int64)
    order = np.argsort(src, kind="stable")
    src_s, dst_s = src[order], dst[order]
    counts = np.bincount(src_s, minlength=n)
    assert counts.max() <= r, f"node degree {counts.max()} > r={r}"
    starts = np.zeros(n, dtype=np.int64)
    starts[1:] = np.cumsum(counts)[:-1]
    slot = (np.arange(len(src_s)) - starts[src_s]) + src_s * r
    dst_pad = np.zeros(n * r, dtype=np.int16)
    mask_pad = np.zeros(n * r, dtype=np.float32)
    dst_pad[slot] = dst_s.astype(np.int16)
    mask_pad[slot] = 1.0
    idx_list, mask_list = [], []
    pad_to = nt_pad * P
    for c in range(cfg.ncores):
        dp = np.pad(dst_pad[c * nloc * r:(c + 1) * nloc * r],
                    (0, pad_to - nloc * r))
        mp = np.pad(mask_pad[c * nloc * r:(c + 1) * nloc * r],
                    (0, pad_to - nloc * r))
        # flat position j = T*128 + p, p = node_in_tile*r + slot
        idx_w = np.tile(dp.reshape(-1, 16).T, (8, 1))  # [128, nt_pad*8]
        mtiles = mp.reshape(nt_pad, P).T.copy()        # [128, nt_pad]
        idx_list.append(np.ascontiguousarray(idx_w, dtype=np.int16))
        mask_list.append(np.ascontiguousarray(mtiles, dtype=np.float32))
    return idx_list, mask_list, counts


def _to_bf16(a):
    import ml_dtypes
    return np.asarray(a, dtype=np.float32).astype(ml_dtypes.bfloat16)


def _seg_mats(cfg: Cfg):
    p = np.arange(P)
    tpb = cfg.tpb
    sseg = np.zeros((P, tpb, P), np.float32)
    for k in range(tpb):
        sseg[p, k, cfg.npt * k + p // cfg.r] = 1.0
    sseg = sseg.reshape(P, tpb * P)
    m8 = np.zeros((P, 8, P), np.float32)
    for k in range(8):
        m8[np.arange(P), k, :] = 0.0
        for e in range(P):
            m8[k * 16 + e // cfg.r, k, e] = 1.0
    m8 = m8.reshape(P, 8 * P)
    return sseg, m8


_PROG_CACHE: dict = {}


def get_program(cfg: Cfg):
    if cfg not in _PROG_CACHE:
        _PROG_CACHE[cfg] = build_program(cfg)
    return _PROG_CACHE[cfg]


def make_cfg(inputs, **overrides) -> Cfg:
    gamma1 = np.asarray(inputs["gamma1"], np.float32)
    beta1 = np.asarray(inputs["beta1"], np.float32)
    gamma2 = np.asarray(inputs["gamma2"], np.float32)
    beta2 = np.asarray(inputs["beta2"], np.float32)
    bqkv = np.concatenate([np.asarray(inputs["bq"], np.float32),
                           np.asarray(inputs["bk"], np.float32),
                           np.asarray(inputs["bv"], np.float32)])
    bo = np.asarray(inputs["bo"], np.float32)
    edges = np.asarray(inputs["edges"])
    n = overrides.get("n", N)
    counts = np.bincount(np.asarray(edges[:, 0], np.int64), minlength=n)
    r = 8
    while r < counts.max():
        r *= 2
    assert r <= P
    return Cfg(
        r=r,
        mask_all=bool(counts.min() < r),
        apply_gb1=not (np.all(gamma1 == 1) and np.all(beta1 == 0)),
        apply_gb2=not (np.all(gamma2 == 1) and np.all(beta2 == 0)),
        apply_bqkv=bool(np.any(bqkv != 0)),
        apply_bo=bool(np.any(bo != 0)),
        **overrides,
    )


def make_in_maps(inputs: dict, cfg: Cfg):
    import ml_dtypes
    x = np.asarray(inputs["x"], dtype=np.float32)
    edges = np.asarray(inputs["edges"])
    s = 1.0 / math.sqrt(DH)
    wqkv = np.concatenate([np.asarray(inputs["wq"], np.float32) * s,
                           np.asarray(inputs["wk"], np.float32),
                           np.asarray(inputs["wv"], np.float32)], axis=1)
    wo = np.asarray(inputs["wo"], np.float32)
    bqkv = np.concatenate([np.asarray(inputs["bq"], np.float32) * s,
                           np.asarray(inputs["bk"], np.float32),
                           np.asarray(inputs["bv"], np.float32)])
    bo = np.asarray(inputs["bo"], np.float32)
    gb = np.stack([np.asarray(inputs["gamma1"], np.float32),
                   np.asarray(inputs["beta1"], np.float32),
                   np.asarray(inputs["gamma2"], np.float32),
                   np.asarray(inputs["beta2"], np.float32)])

    idx_list, mask_list, _ = _prep_edges(edges, cfg)
    sseg, m8 = _seg_mats(cfg)
    brow = np.concatenate([bqkv, bo])[None, :].astype(np.float32)

    wqkv_b = _to_bf16(wqkv)
    wo_b = _to_bf16(wo)
    sseg_b = _to_bf16(sseg)
    m8_b = _to_bf16(m8)
    in_maps = []
    for c in range(cfg.ncores):
        lo, hi = c * cfg.nloc, (c + 1) * cfg.nloc
        x_loc = np.ascontiguousarray(x[:, lo:hi, :].reshape(cfg.b * cfg.nloc, D))
        in_maps.append({
            "x": x_loc,
            "wqkv": wqkv_b,
            "wo": wo_b,
            "idx": idx_list[c],
            "mask": mask_list[c],
            "sseg": sseg_b,
            "m8": m8_b,
            "gb": gb,
            "brow": brow,
        })
    return in_maps


def assemble_out(results, cfg: Cfg):
    out = np.empty((cfg.b, cfg.n, D), dtype=np.float32)
    for c in range(cfg.ncores):
        o = np.asarray(results[c]["out"]).reshape(cfg.b, cfg.nloc, D)
        out[:, c * cfg.nloc:(c + 1) * cfg.nloc, :] = o
    return out


LAST_RESULT = None  # BassKernelResults of the most recent kernel() call


def kernel(**inputs) -> np.ndarray:
    global LAST_RESULT
    from concourse.bass_utils import run_bass_kernel_spmd

    cfg = make_cfg(inputs)
    nc = get_program(cfg)
    in_maps = make_in_maps(inputs, cfg)
    LAST_RESULT = run_bass_kernel_spmd(nc, in_maps, list(range(cfg.ncores)))
    return assemble_out(LAST_RESULT.results, cfg)


# revision 22
# speedup vs baseline: 1.1350x; 1.0170x over previous
"""Sparse (graph-edge) multi-head attention block on 8 TRN2 NeuronCores.

Problem: nn_MultiHeadAttention_6966436954266
  B=2, N=20000, D=256, H=8, dh=32, E=160000 (8 out-edges per node, sorted by src)

  xn  = LN1(x); q,k,v = xn @ w{q,k,v}; per-edge w = exp(q_src.k_dst/sqrt(dh))
  attn = segment_sum(w*v_dst)/segment_sum(w); concat = xn + attn
  out = relu(LN2(concat) @ wo + bo) + concat

Sharding: nodes partitioned contiguously across 8 cores (2500 each). Each core
LN+projects its shard into an fp8 K/V row table (row = [K0 V0 K1 V1], 1KB),
AllGathered into one Shared 20MB table; each core then runs the edge stage for
its own nodes' edges (grouped by src, 8 slots/node).

Edge-stage per 128-node block (8 tiles of 16 nodes x 8 slots):
  - ONE dma_gather per 512 edge slots fetches the full 1KB row (K and V for
    both batches) -> [128, 4, 1024] fp8. SWDGE descriptor emission (~8.6ns per
    descriptor, measured) is the gather bottleneck, so descriptor count is
    minimized by fetching everything an edge needs in one descriptor.
  - q rows are broadcast node->edge-slots with a constant [16,128] selector
    matmul on PE (out = M16^T @ q_tile), PSUM f32.
  - qk product on DVE (fp8 K upconverts in-op), per-head logits via strided
    tensor_reduce, exp on ACT (wq/bq are pre-scaled by 1/sqrt(dh) on host).
  - w*v on DVE with free-dim broadcast; segment sum via PE (lhsT = constant
    0/1 slot-selector), accumulating [128 nodes, 256+8] in PSUM per block.
  - Block epilogue (attn=num/den, concat=xn+attn, LN2 + wo matmul + relu +
    residual + store) runs fused, overlapping later blocks' gathers.
"""

import math
from dataclasses import dataclass

import numpy as np

import concourse.bass as bass
import concourse.bacc as bacc
import concourse.mybir as mybir
import concourse.tile as tile
from concourse.masks import make_identity

B, N, D, H, DH = 2, 20000, 256, 8, 32
NCORES = 8
EPS = 1e-3
P = 128
F32 = mybir.dt.float32
BF16 = mybir.dt.bfloat16
FP8 = mybir.dt.float8e4
I16 = mybir.dt.int16
AF = mybir.ActivationFunctionType
ALU = mybir.AluOpType
SUBT = 4  # tiles per dma_gather (512-index SWDGE ring limit)


@dataclass(frozen=True)
class Cfg:
    n: int = N
    nloc: int = N // NCORES
    ncores: int = NCORES
    b: int = B
    r: int = 8          # edge slots per node (pow2, divides 128)
    mask_all: bool = False
    apply_gb1: bool = False
    apply_gb2: bool = False
    apply_bqkv: bool = False
    apply_bo: bool = False
    swdge_queues: int = 4

    @property
    def npt(self):  # nodes per 128-edge tile
        return P // self.r

    @property
    def nt(self):  # real edge tiles per batch
        return math.ceil(self.nloc / self.npt)

    @property
    def tpb(self):  # tiles per 128-node block
        return P // self.npt

    @property
    def nblk(self):  # 128-node blocks (gather/segment granularity)
        return math.ceil(self.nt / self.tpb)

    @property
    def nt_pad(self):  # idx tiles padded to whole blocks
        return self.nblk * self.tpb

    @property
    def rt(self):  # 128-row tiles per batch (dense stages)
        return math.ceil(self.nloc / P)


def _ceil_div(a, b):
    return (a + b - 1) // b


# ------------------------------------------------------------------- program
def build_program(cfg: Cfg) -> bass.Bass:
    nc = bacc.Bacc("TRN2", num_devices=cfg.ncores,
                   num_swdge_queues=cfg.swdge_queues)
    d = D
    rt = cfg.rt
    nloc, b_ = cfg.nloc, cfg.b

    x_in = nc.dram_tensor("x", [b_ * nloc, d], F32, kind="ExternalInput")
    wqkv_in = nc.dram_tensor("wqkv", [d, 3 * d], BF16, kind="ExternalInput")
    wo_in = nc.dram_tensor("wo", [d, d], BF16, kind="ExternalInput")
    idx_in = nc.dram_tensor("idx", [P, cfg.nt_pad * (P // 16)], I16,
                            kind="ExternalInput")
    mask_in = nc.dram_tensor("mask", [P, cfg.nt_pad], F32, kind="ExternalInput")
    sseg_in = nc.dram_tensor("sseg", [P, cfg.tpb * P], BF16,
                             kind="ExternalInput")
    m8_in = nc.dram_tensor("m8", [P, 8 * P], BF16, kind="ExternalInput")
    gb_in = nc.dram_tensor("gb", [4, d], F32, kind="ExternalInput")
    brow_in = nc.dram_tensor("brow", [1, 4 * d], F32, kind="ExternalInput")
    out_ext = nc.dram_tensor("out", [b_ * nloc, d], F32, kind="ExternalOutput")

    kv_loc = [nc.dram_tensor(f"kv_loc{b}", [nloc, 2 * d], FP8)
              for b in range(b_)]
    kv_full = [nc.dram_tensor(
        f"kv_full{b}", [cfg.n, 2 * d], FP8,
        addr_space="Shared" if cfg.ncores > 4 else "Local")
        for b in range(b_)]

    with tile.TileContext(nc) as tc:
        with tc.tile_pool(name="const", bufs=1) as cpool:
            ident_f = cpool.tile([P, P], F32)
            make_identity(nc, ident_f[:])
            ident_b = cpool.tile([P, P], BF16)
            make_identity(nc, ident_b[:])
            wqkv_sb = cpool.tile([P, 2, 3 * d], BF16)
            nc.sync.dma_start(
                out=wqkv_sb[:],
                in_=wqkv_in[:].rearrange("(ci p) q -> p ci q", p=P))
            wo_sb = cpool.tile([P, 2, d], BF16)
            nc.sync.dma_start(
                out=wo_sb[:], in_=wo_in[:].rearrange("(ci p) q -> p ci q", p=P))
            sseg_sb = cpool.tile([P, cfg.tpb, P], BF16)
            nc.sync.dma_start(
                out=sseg_sb[:],
                in_=sseg_in[:].rearrange("p (k m) -> p k m", k=cfg.tpb))
            m8_sb = cpool.tile([P, 8, P], BF16)
            nc.sync.dma_start(
                out=m8_sb[:], in_=m8_in[:].rearrange("p (k m) -> p k m", k=8))
            idx_sb = cpool.tile([P, cfg.nt_pad * (P // 16)], I16)
            nc.sync.dma_start(out=idx_sb[:], in_=idx_in[:])
            mask_sb = cpool.tile([P, cfg.nt_pad], F32)
            if cfg.mask_all or cfg.nloc % cfg.npt:
                nc.sync.dma_start(out=mask_sb[:], in_=mask_in[:])
            eps_sb = cpool.tile([P, 1], F32)
            nc.vector.memset(eps_sb[:], EPS)
            gb_sb = cpool.tile([1, 4, d], F32)
            if cfg.apply_gb1 or cfg.apply_gb2:
                nc.sync.dma_start(out=gb_sb[:],
                                  in_=gb_in[:].rearrange("g d -> 1 g d"))
            brow_sb = cpool.tile([1, 4 * d], F32)
            if cfg.apply_bqkv or cfg.apply_bo:
                nc.sync.dma_start(out=brow_sb[:], in_=brow_in[:])

            with tc.tile_pool(name="resident", bufs=1) as rpool:
                xn_sb = rpool.tile([P, b_ * rt, d], BF16)
                xnt_sb = rpool.tile([P, 2, b_ * rt, P], BF16)
                q_sb = rpool.tile([P, b_, rt, d], BF16)
                for b in range(b_):
                    _stage1(nc, tc, cfg, x_in, xn_sb, xnt_sb, ident_b, eps_sb,
                            gb_sb, b)
                    _stage2_kv(nc, tc, cfg, xnt_sb, wqkv_sb, brow_sb,
                               kv_loc[b], b)
                    nc.gpsimd.collective_compute(
                        "AllGather",
                        ALU.bypass,
                        replica_groups=[list(range(cfg.ncores))],
                        ins=[kv_loc[b][:]],
                        outs=[kv_full[b][:]],
                    )
                _stage2_q(nc, tc, cfg, xnt_sb, wqkv_sb, brow_sb, q_sb)
                for b in range(b_):
                    _stage4(nc, tc, cfg, idx_sb, kv_full[b], q_sb, xn_sb,
                            sseg_sb, m8_sb, mask_sb, wo_sb, ident_b, eps_sb,
                            gb_sb, brow_sb, out_ext, b)
                    _stage5(nc, tc, cfg, xn_sb, wo_sb, ident_b, eps_sb, gb_sb,
                            brow_sb, out_ext, b)
    nc.finalize()
    return nc


def _layer_norm_rs(nc, pool, src_ap, rows, eps_sb):
    """bn_stats -> mv [P,2] f32 with [:,0]=mean, [:,1]=1/sqrt(var+eps)."""
    stats = pool.tile([P, 6], F32, tag="ln_stats")
    nc.vector.bn_stats(out=stats[:rows], in_=src_ap)
    mv = pool.tile([P, 2], F32, tag="ln_mv")
    nc.vector.bn_aggr(out=mv[:rows], in_=stats[:rows])
    nc.scalar.activation(out=mv[:rows, 1:2], in_=mv[:rows, 1:2], func=AF.Sqrt,
                         bias=eps_sb[:rows], scale=1.0)
    nc.vector.reciprocal(out=mv[:rows, 1:2], in_=mv[:rows, 1:2])
    return mv


def _stage1(nc, tc, cfg, x_in, xn_sb, xnt_sb, ident_b, eps_sb, gb_sb, b):
    d, rt, nloc = D, cfg.rt, cfg.nloc
    with tc.tile_pool(name=f"s1_{b}", bufs=6) as pool, \
         tc.tile_pool(name=f"s1p_{b}", bufs=6, space="PSUM") as ppool:
        if True:
            for irt in range(rt):
                bt = b * rt + irt
                rows = min(P, nloc - irt * P)
                xt = pool.tile([P, d], F32, tag="xt")
                nc.sync.dma_start(
                    out=xt[:rows],
                    in_=x_in[b * nloc + irt * P: b * nloc + irt * P + rows, :])
                if rows < P:
                    nc.vector.memset(xn_sb[:, bt, :], 0.0)
                mv = _layer_norm_rs(nc, pool, xt[:rows], rows, eps_sb)
                nc.vector.tensor_scalar(
                    out=xn_sb[:rows, bt, :], in0=xt[:rows],
                    scalar1=mv[:rows, 0:1], scalar2=mv[:rows, 1:2],
                    op0=ALU.subtract, op1=ALU.mult)
                if cfg.apply_gb1:
                    nc.vector.tensor_tensor(
                        out=xn_sb[:rows, bt, :], in0=xn_sb[:rows, bt, :],
                        in1=gb_sb[:, 0, :].partition_broadcast(rows),
                        op=ALU.mult)
                    nc.vector.tensor_tensor(
                        out=xn_sb[:rows, bt, :], in0=xn_sb[:rows, bt, :],
                        in1=gb_sb[:, 1, :].partition_broadcast(rows),
                        op=ALU.add)
                for ci in range(2):
                    pt = ppool.tile([P, P], BF16, tag="tr")
                    nc.tensor.transpose(
                        out=pt[:], in_=xn_sb[:, bt, ci * P:(ci + 1) * P],
                        identity=ident_b[:])
                    nc.scalar.copy(out=xnt_sb[:, ci, bt, :], in_=pt[:])


def _stage2_kv(nc, tc, cfg, xnt_sb, wqkv_sb, brow_sb, kv_loc, b):
    """K,V projections -> fp8 rows [K_b | V_b] for one batch."""
    d, rt, nloc = D, cfg.rt, cfg.nloc
    with tc.tile_pool(name=f"s2kv_{b}", bufs=4) as pool, \
         tc.tile_pool(name=f"s2kvp_{b}", bufs=2, space="PSUM") as ppool:
        if True:
            for irt in range(rt):
                bt = b * rt + irt
                rows = min(P, nloc - irt * P)
                ps = ppool.tile([P, 2 * d], F32, tag="kv")
                for ci in range(2):
                    nc.tensor.matmul(
                        out=ps[:], lhsT=xnt_sb[:, ci, bt, :],
                        rhs=wqkv_sb[:, ci, d:3 * d],
                        start=(ci == 0), stop=(ci == 1))
                kvb = pool.tile([P, 2 * d], FP8, tag="kvb")
                if cfg.apply_bqkv:
                    nc.vector.tensor_tensor(
                        out=kvb[:rows], in0=ps[:rows],
                        in1=brow_sb[:, d:3 * d].partition_broadcast(rows),
                        op=ALU.add)
                else:
                    nc.scalar.copy(out=kvb[:rows], in_=ps[:rows])
                nc.sync.dma_start(
                    out=kv_loc[irt * P: irt * P + rows, :],
                    in_=kvb[:rows])


def _stage2_q(nc, tc, cfg, xnt_sb, wqkv_sb, brow_sb, q_sb):
    """Q rows (pre-scaled by 1/sqrt(dh) via host-scaled wq) in bf16."""
    d, rt, nloc = D, cfg.rt, cfg.nloc
    with tc.tile_pool(name="s2q", bufs=3) as pool, \
         tc.tile_pool(name="s2qp", bufs=2, space="PSUM") as ppool:
        for b in range(cfg.b):
            for irt in range(rt):
                bt = b * rt + irt
                rows = min(P, nloc - irt * P)
                ps = ppool.tile([P, d], F32, tag="q")
                for ci in range(2):
                    nc.tensor.matmul(
                        out=ps[:], lhsT=xnt_sb[:, ci, bt, :],
                        rhs=wqkv_sb[:, ci, 0:d],
                        start=(ci == 0), stop=(ci == 1))
                if rows < P:
                    nc.vector.memset(q_sb[:, b, irt, :], 0.0)
                if cfg.apply_bqkv:
                    nc.vector.tensor_tensor(
                        out=q_sb[:rows, b, irt, :], in0=ps[:rows],
                        in1=brow_sb[:, 0:d].partition_broadcast(rows),
                        op=ALU.add)
                else:
                    nc.scalar.copy(out=q_sb[:rows, b, irt, :], in_=ps[:rows])


def _stage4(nc, tc, cfg, idx_sb, kv_full, q_sb, xn_sb, sseg_sb, m8_sb,
            mask_sb, wo_sb, ident_b, eps_sb, gb_sb, brow_sb, out_ext, b):
    d, rt, nloc = D, cfg.rt, cfg.nloc
    npt, tpb, nblk = cfg.npt, cfg.tpb, cfg.nblk
    row_elems = 2 * d  # fp8 elements per kv row
    cpt = P // 16              # idx columns per 128-edge tile
    spb = tpb // SUBT          # sub-gathers per block
    first_pad_tile = 0 if cfg.mask_all else \
        ((nloc // npt) if nloc % npt else cfg.nt)

    with tc.tile_pool(name=f"s4g_{b}", bufs=3) as gpool, \
         tc.tile_pool(name=f"s4t_{b}", bufs=3) as tpool, \
         tc.tile_pool(name=f"s4pq_{b}", bufs=1, space="PSUM") as pqpool, \
         tc.tile_pool(name=f"s4pb_{b}", bufs=2, space="PSUM") as pbpool:
        for blk in range(nblk):
            buf = gpool.tile([P, spb, SUBT, row_elems], FP8, tag="rows")
            for j in range(spb):
                t0 = blk * tpb + j * SUBT
                nc.gpsimd.dma_gather(
                    out_ap=buf[:, j, :, :],
                    in_ap=kv_full[:, :],
                    idxs_ap=idx_sb[:, t0 * cpt: (t0 + SUBT) * cpt],
                    num_idxs=SUBT * P, num_idxs_reg=SUBT * P,
                    elem_size=row_elems, elem_step=row_elems,
                    queue_num=(blk * spb + j) % cfg.swdge_queues)
            bufk = tpool.tile([P, spb, SUBT, d], BF16, tag="bufk")
            nc.scalar.copy(out=bufk[:], in_=buf[:, :, :, 0:d])
            bufv = tpool.tile([P, spb, SUBT, d], BF16, tag="bufv")
            nc.gpsimd.dma_start(out=bufv[:], in_=buf[:, :, :, d:2 * d])
            nb = pbpool.tile([P, d + 8], F32, tag="nb")
            qrep = pqpool.tile([P, tpb, d], F32, tag="qrep")
            for sl in range(tpb):
                nc.tensor.matmul(
                    out=qrep[:, sl, :], lhsT=m8_sb[:, sl, :],
                    rhs=q_sb[:, b, blk, :],
                    start=True, stop=True)
            qrep_sb = tpool.tile([P, tpb, d], BF16, tag="qrepsb")
            nc.scalar.copy(out=qrep_sb[:], in_=qrep[:])
            qk = tpool.tile([P, tpb, d], BF16, tag="qk")
            nc.vector.tensor_tensor(
                out=qk[:].rearrange("p (j t) q -> p j t q", j=spb),
                in0=qrep_sb[:].rearrange("p (j t) q -> p j t q", j=spb),
                in1=bufk[:, :, :, :],
                op=ALU.mult)
            qkh = tpool.tile([P, tpb, 8, DH // 2], BF16, tag="qkh")
            nc.vector.tensor_tensor(
                out=qkh[:],
                in0=qk[:].rearrange("p t (h x) -> p t h x", x=DH)[:, :, :, 0:DH // 2],
                in1=qk[:].rearrange("p t (h x) -> p t h x", x=DH)[:, :, :, DH // 2:DH],
                op=ALU.add)
            lg = tpool.tile([P, tpb, 8], F32, tag="lg")
            nc.vector.tensor_reduce(
                out=lg[:], in_=qkh[:],
                axis=mybir.AxisListType.X, op=ALU.add)
            rhs_t = tpool.tile([P, tpb, d + 8], BF16, tag="rhs")
            nc.scalar.activation(
                out=rhs_t[:, :, d:d + 8], in_=lg[:], func=AF.Exp,
                scale=1.0)
            t_first = blk * tpb
            if t_first + tpb > first_pad_tile:
                nc.vector.tensor_tensor(
                    out=rhs_t[:, :, d:d + 8],
                    in0=rhs_t[:, :, d:d + 8],
                    in1=mask_sb[:, t_first:t_first + tpb]
                    .unsqueeze(-1).broadcast_to([P, tpb, 8]),
                    op=ALU.mult)
            nc.vector.tensor_tensor(
                out=rhs_t[:, :, :d].rearrange("p t (h x) -> p t h x", x=DH),
                in0=bufv[:, :, :, :]
                .rearrange("p j t q -> p (j t) q")
                .rearrange("p t (h x) -> p t h x", x=DH),
                in1=rhs_t[:, :, d:d + 8]
                .unsqueeze(-1).broadcast_to([P, tpb, 8, DH]),
                op=ALU.mult)
            for sl in range(tpb):
                nc.tensor.matmul(
                    out=nb[:], lhsT=sseg_sb[:, sl, :],
                    rhs=rhs_t[:, sl, :],
                    start=(sl == 0), stop=(sl == tpb - 1))
            base = blk * P
            valid = min(P, nloc - base)
            bt = b * rt + blk
            rec = tpool.tile([P, 8], F32, tag="rec")
            nc.vector.reciprocal(out=rec[:valid], in_=nb[:valid, d:d + 8])
            tmp = tpool.tile([P, d], BF16, tag="attn")
            nc.vector.tensor_tensor(
                out=tmp[:valid].rearrange("p (h x) -> p h x", x=DH),
                in0=nb[:valid, :d].rearrange("p (h x) -> p h x", x=DH),
                in1=rec[:valid].unsqueeze(-1).broadcast_to([valid, H, DH]),
                op=ALU.mult)
            nc.vector.tensor_tensor(
                out=xn_sb[:valid, bt, :], in0=xn_sb[:valid, bt, :],
                in1=tmp[:valid], op=ALU.add)


def _stage5(nc, tc, cfg, xn_sb, wo_sb, ident_b, eps_sb, gb_sb, brow_sb,
            out_ext, b):
    with tc.tile_pool(name=f"s5_{b}", bufs=4) as pool, \
         tc.tile_pool(name=f"s5o_{b}", bufs=3) as opool, \
         tc.tile_pool(name=f"s5p_{b}", bufs=2, space="PSUM") as ppool:
        for blk in range(cfg.nblk):
            valid = min(P, cfg.nloc - blk * P)
            _stage5_block(nc, cfg, pool, opool, ppool, xn_sb, wo_sb, ident_b,
                          eps_sb, gb_sb, brow_sb, out_ext, b, blk, valid)


def _stage5_block(nc, cfg, pool, opool, ppool, xn_sb, wo_sb, ident_b, eps_sb,
                  gb_sb, brow_sb, out_ext, b, blk, rows):
    """Fused output stage for one 128-row block (concat rows are final)."""
    d, rt, nloc = D, cfg.rt, cfg.nloc
    bt = b * rt + blk
    mv = _layer_norm_rs(nc, pool, xn_sb[:rows, bt, :], rows, eps_sb)
    cnb = pool.tile([P, d], BF16, tag="cnb")
    if rows < P:
        nc.vector.memset(cnb[:], 0.0)
    if cfg.apply_gb2:
        cn32 = pool.tile([P, d], F32, tag="cn32")
        nc.vector.tensor_scalar(
            out=cn32[:rows], in0=xn_sb[:rows, bt, :],
            scalar1=mv[:rows, 0:1], scalar2=mv[:rows, 1:2],
            op0=ALU.subtract, op1=ALU.mult)
        nc.vector.tensor_tensor(
            out=cn32[:rows], in0=cn32[:rows],
            in1=gb_sb[:, 2, :].partition_broadcast(rows), op=ALU.mult)
        nc.vector.tensor_tensor(
            out=cnb[:rows], in0=cn32[:rows],
            in1=gb_sb[:, 3, :].partition_broadcast(rows), op=ALU.add)
    else:
        nc.vector.tensor_scalar(
            out=cnb[:rows], in0=xn_sb[:rows, bt, :],
            scalar1=mv[:rows, 0:1], scalar2=mv[:rows, 1:2],
            op0=ALU.subtract, op1=ALU.mult)
    y = ppool.tile([P, d], F32, tag="y")
    for ci in range(2):
        pt = ppool.tile([P, P], BF16, tag="tr5")
        nc.tensor.transpose(out=pt[:], in_=cnb[:, ci * P:(ci + 1) * P],
                            identity=ident_b[:])
        cnt = pool.tile([P, P], BF16, tag="cnt")
        nc.vector.tensor_copy(out=cnt[:], in_=pt[:])
        nc.tensor.matmul(out=y[:], lhsT=cnt[:], rhs=wo_sb[:, ci, :],
                         start=(ci == 0), stop=(ci == 1))
    if cfg.apply_bo:
        nc.vector.tensor_tensor(
            out=y[:rows], in0=y[:rows],
            in1=brow_sb[:, 3 * d: 4 * d].partition_broadcast(rows),
            op=ALU.add)
    ot = opool.tile([P, d], F32, tag="ot")
    nc.vector.scalar_tensor_tensor(
        out=ot[:rows], in0=y[:rows], scalar=0.0, in1=xn_sb[:rows, bt, :],
        op0=ALU.max, op1=ALU.add)
    nc.sync.dma_start(
        out=out_ext[b * nloc + blk * P: b * nloc + blk * P + rows, :],
        in_=ot[:rows])


# ------------------------------------------------------------------ host side
def _prep_edges(edges: np.ndarray, cfg: Cfg):
    """Group edges by src, pad each node to cfg.r slots.

    Returns per-core lists: idx [128, nt_pad*8] int16, mask [128, nt_pad] f32.
    """
    n, nloc, r, nt_pad = cfg.n, cfg.nloc, cfg.r, cfg.nt_pad
    src = np.asarray(edges[:, 0], dtype=np.int64)
    dst = np.asarray(edges[:, 1], dtype=np.int64)
    order = np.argsort(src, kind="stable")
    src_s, dst_s = src[order], dst[order]
    counts = np.bincount(src_s, minlength=n)
    assert counts.max() <= r, f"node degree {counts.max()} > r={r}"
    starts = np.zeros(n, dtype=np.int64)
    starts[1:] = np.cumsum(counts)[:-1]
    slot = (np.arange(len(src_s)) - starts[src_s]) + src_s * r
    dst_pad = np.zeros(n * r, dtype=np.int16)
    mask_pad = np.zeros(n * r, dtype=np.float32)
    dst_pad[slot] = dst_s.astype(np.int16)
    mask_pad[slot] = 1.0
    idx_list, mask_list = [], []
    pad_to = nt_pad * P
    for c in range(cfg.ncores):
        dp = np.pad(dst_pad[c * nloc * r:(c + 1) * nloc * r],
                    (0, pad_to - nloc * r))
        mp = np.pad(mask_pad[c * nloc * r:(c + 1) * nloc * r],
                    (0, pad_to - nloc * r))
        # flat position j = T*128 + p, p = node_in_tile*r + slot
        idx_w = np.tile(dp.reshape(-1, 16).T, (8, 1))  # [128, nt_pad*8]
        mtiles = mp.reshape(nt_pad, P).T.copy()        # [128, nt_pad]
        idx_list.append(np.ascontiguousarray(idx_w, dtype=np.int16))
        mask_list.append(np.ascontiguousarray(mtiles, dtype=np.float32))
    return idx_list, mask_list, counts


def _to_bf16(a):
    import ml_dtypes
    return np.asarray(a, dtype=np.float32).astype(ml_dtypes.bfloat16)


def _seg_mats(cfg: Cfg):
    p = np.arange(P)
    tpb = cfg.tpb
    sseg = np.zeros((P, tpb, P), np.float32)
    for k in range(tpb):
        sseg[p, k, cfg.npt * k + p // cfg.r] = 1.0
    sseg = sseg.reshape(P, tpb * P)
    m8 = np.zeros((P, 8, P), np.float32)
    for k in range(8):
        m8[np.arange(P), k, :] = 0.0
        for e in range(P):
            m8[k * 16 + e // cfg.r, k, e] = 1.0
    m8 = m8.reshape(P, 8 * P)
    return sseg, m8


_PROG_CACHE: dict = {}


def get_program(cfg: Cfg):
    if cfg not in _PROG_CACHE:
        _PROG_CACHE[cfg] = build_program(cfg)
    return _PROG_CACHE[cfg]


def make_cfg(inputs, **overrides) -> Cfg:
    gamma1 = np.asarray(inputs["gamma1"], np.float32)
    beta1 = np.asarray(inputs["beta1"], np.float32)
    gamma2 = np.asarray(inputs["gamma2"], np.float32)
    beta2 = np.asarray(inputs["beta2"], np.float32)
    bqkv = np.concatenate([np.asarray(inputs["bq"], np.float32),
                           np.asarray(inputs["bk"], np.float32),
                           np.asarray(inputs["bv"], np.float32)])
    bo = np.asarray(inputs["bo"], np.float32)
    edges = np.asarray(inputs["edges"])
    n = overrides.get("n", N)
    counts = np.bincount(np.asarray(edges[:, 0], np.int64), minlength=n)
    r = 8
    while r < counts.max():
        r *= 2
    assert r <= P
    return Cfg(
        r=r,
        mask_all=bool(counts.min() < r),
        apply_gb1=not (np.all(gamma1 == 1) and np.all(beta1 == 0)),
        apply_gb2=not (np.all(gamma2 == 1) and np.all(beta2 == 0)),
        apply_bqkv=bool(np.any(bqkv != 0)),
        apply_bo=bool(np.any(bo != 0)),
        **overrides,
    )


def make_in_maps(inputs: dict, cfg: Cfg):
    import ml_dtypes
    x = np.asarray(inputs["x"], dtype=np.float32)
    edges = np.asarray(inputs["edges"])
    s = 1.0 / math.sqrt(DH)
    wqkv = np.concatenate([np.asarray(inputs["wq"], np.float32) * s,
                           np.asarray(inputs["wk"], np.float32),
                           np.asarray(inputs["wv"], np.float32)], axis=1)
    wo = np.asarray(inputs["wo"], np.float32)
    bqkv = np.concatenate([np.asarray(inputs["bq"], np.float32) * s,
                           np.asarray(inputs["bk"], np.float32),
                           np.asarray(inputs["bv"], np.float32)])
    bo = np.asarray(inputs["bo"], np.float32)
    gb = np.stack([np.asarray(inputs["gamma1"], np.float32),
                   np.asarray(inputs["beta1"], np.float32),
                   np.asarray(inputs["gamma2"], np.float32),
                   np.asarray(inputs["beta2"], np.float32)])

    idx_list, mask_list, _ = _prep_edges(edges, cfg)
    sseg, m8 = _seg_mats(cfg)
    brow = np.concatenate([bqkv, bo])[None, :].astype(np.float32)

    wqkv_b = _to_bf16(wqkv)
    wo_b = _to_bf16(wo)
    sseg_b = _to_bf16(sseg)
    m8_b = _to_bf16(m8)
    in_maps = []
    for c in range(cfg.ncores):
        lo, hi = c * cfg.nloc, (c + 1) * cfg.nloc
        x_loc = np.ascontiguousarray(x[:, lo:hi, :].reshape(cfg.b * cfg.nloc, D))
        in_maps.append({
            "x": x_loc,
            "wqkv": wqkv_b,
            "wo": wo_b,
            "idx": idx_list[c],
            "mask": mask_list[c],
            "sseg": sseg_b,
            "m8": m8_b,
            "gb": gb,
            "brow": brow,
        })
    return in_maps


def assemble_out(results, cfg: Cfg):
    out = np.empty((cfg.b, cfg.n, D), dtype=np.float32)
    for c in range(cfg.ncores):
        o = np.asarray(results[c]["out"]).reshape(cfg.b, cfg.nloc, D)
        out[:, c * cfg.nloc:(c + 1) * cfg.nloc, :] = o
    return out


LAST_RESULT = None  # BassKernelResults of the most recent kernel() call


def kernel(**inputs) -> np.ndarray:
    global LAST_RESULT
    from concourse.bass_utils import run_bass_kernel_spmd

    cfg = make_cfg(inputs)
    nc = get_program(cfg)
    in_maps = make_in_maps(inputs, cfg)
    LAST_RESULT = run_bass_kernel_spmd(nc, in_maps, list(range(cfg.ncores)))
    return assemble_out(LAST_RESULT.results, cfg)
